# revision 12
# baseline (speedup 1.0000x reference)
"""Distributed Trainium2 (8 NeuronCores) kernel for nn_AdaptiveAttention.

Reference computation (b=2, n=2048, d=1024, 16 heads x 64):
    qkv = x @ W_qkv; q,k,v = split(qkv)
    attn = softmax(mask(q k^T / sqrt(dh)))
    out  = (attn @ v) @ W_out + b_out

Sharding: core c in [0,8) handles batch b = c//4 and head group g = c%4
(heads 4g..4g+3).  Data parallel over b, tensor parallel over heads.
The output projection needs all heads, so the per-head attention outputs
(plus softmax denominators) are AllGather'd (bf16, groups of 4) and each
core computes the output projection for its quarter of the sequence,
selected with a dynamic (partition-id-derived) slice.

Numerics: projections and scores run the TensorEngine in float32r
(fp32 storage, ~1e-4 matmul error, full PE rate); exp output, mask and
the P = exp(s)*mask matrix are bf16; all matmuls accumulate in fp32.
Softmax runs without max-subtraction (scores are O(1) by construction)
as exp(s) * mask, with row sums obtained by augmenting v with a ones
column; normalization happens after the AllGather via a reciprocal and
a small expansion matmul.
"""

import numpy as np
import ml_dtypes

import concourse.bass as bass
import concourse.tile as tile
from concourse import bacc, mybir
from concourse import bass_utils

BF16 = ml_dtypes.bfloat16

B = 2
N = 2048
D = 1024
HEADS = 16
HD = 64  # head dim
SCALE = HD ** -0.5
N_CORES = 8
HPC = 4  # heads per core
IB = 1024  # i-block size in attention inner tiles
NJ = N // 128  # 16 j-chunks

_cached_nc = None
_last_in_maps = None
_last_res = None
DEBUG = False


def _build():
    nc = bacc.Bacc("TRN2", target_bir_lowering=False, debug=False,
                   num_devices=N_CORES)

    f32 = mybir.dt.float32
    f32r = mybir.dt.float32r
    bf = mybir.dt.bfloat16

    xt = nc.dram_tensor("xt", [D, N], f32, kind="ExternalInput")
    wqkv = nc.dram_tensor("wqkv", [D, 768], f32, kind="ExternalInput")
    maskt = nc.dram_tensor("maskt", [N, N], bf, kind="ExternalInput")
    wout = nc.dram_tensor("wout", [D, D], bf, kind="ExternalInput")
    emat = nc.dram_tensor("emat", [HEADS, D], bf, kind="ExternalInput")
    out = nc.dram_tensor("out", [N // 4, D], f32, kind="ExternalOutput")
    if DEBUG:
        dbg_qkt = nc.dram_tensor("dbg_qkt", [128, 4 * N], f32,
                                 kind="ExternalOutput")
        dbg_vaug = nc.dram_tensor("dbg_vaug", [128, NJ * 260], bf,
                                  kind="ExternalOutput")
        dbg_ag = nc.dram_tensor("dbg_ag", [4 * 4 * 65, N], bf,
                                kind="ExternalOutput")
        dbg_sums = nc.dram_tensor("dbg_sums", [HEADS, 512], bf,
                                  kind="ExternalOutput")
        dbg_rec = nc.dram_tensor("dbg_rec", [HEADS, 512], bf,
                                 kind="ExternalOutput")
        dbg_attnn = nc.dram_tensor("dbg_attnn", [128, 8 * 512], bf,
                                   kind="ExternalOutput")

    with tile.TileContext(nc) as tc:
        with (
            tc.tile_pool(name="res", bufs=1) as res,
            tc.tile_pool(name="dram", bufs=1, space="DRAM") as dram,
        ):
            # resident tensors
            # qkt: [qT01 | qT23 | kT01 | kT23], each [128, 2048] fp32r
            qkt = res.tile([128, 4 * N], f32r)
            # v_aug: per j-chunk jc block of 260 cols: 4x(64 v cols + ones)
            v_aug = res.tile([128, NJ * 260], bf)

            ag_in = dram.tile([4 * 65, N], bf)       # rows: 4x64 attn + 4 sums
            ag_out = dram.tile([4 * 4 * 65, N], bf)  # gathered over 4 ranks

            # ---------------- phase 0: load + projections ----------------
            with (
                tc.tile_pool(name="ph0", bufs=1) as p0,
                tc.tile_pool(name="pp_qk", bufs=4, space="PSUM") as pp_qk,
                tc.tile_pool(name="pp_v", bufs=2, space="PSUM") as pp_v,
            ):
                xtr = p0.tile([128, 8 * N], f32r)
                wr = p0.tile([128, 8 * 768], f32r)
                for k in range(8):
                    # DMA-cast f32 -> f32r (SWDGE)
                    nc.gpsimd.dma_start(xtr[:, N * k:N * (k + 1)],
                                        xt[128 * k:128 * (k + 1), :])
                    nc.gpsimd.dma_start(wr[:, 768 * k:768 * (k + 1)],
                                        wqkv[128 * k:128 * (k + 1), :])

                nc.vector.memset(v_aug[:], 1.0)

                # qT / kT stacked pairs: wcol selects the 128 W columns
                for t_i in range(4):  # qT01, qT23, kT01, kT23
                    wcol = 128 * t_i
                    for nb in range(4):  # n blocks of 512
                        ps = pp_qk.tile([128, 512], f32, name="ps_qk", tag="ps_qk")
                        for k in range(8):
                            nc.tensor.matmul(
                                ps[:],
                                wr[:, 768 * k + wcol:768 * k + wcol + 128],
                                xtr[:, N * k + 512 * nb:N * k + 512 * nb + 512],
                                start=(k == 0), stop=(k == 7),
                            )
                        nc.vector.tensor_copy(
                            qkt[:, N * t_i + 512 * nb:N * t_i + 512 * nb + 512],
                            ps[:])

                # v: natural layout [j, 4*64], copied into v_aug with ones col
                for jc in range(NJ):
                    ps = pp_v.tile([128, 256], f32, name="ps_v", tag="ps_v")
                    for k in range(8):
                        nc.tensor.matmul(
                            ps[:],
                            xtr[:, N * k + 128 * jc:N * k + 128 * jc + 128],
                            wr[:, 768 * k + 512:768 * k + 768],
                            start=(k == 0), stop=(k == 7),
                        )
                    dst = v_aug[:, 260 * jc:260 * jc + 260]
                    dst3 = dst.rearrange("p (h c) -> p h c", h=4, c=65)[:, :, 0:64]
                    src3 = ps[:].rearrange("p (h c) -> p h c", h=4, c=64)
                    nc.vector.tensor_copy(dst3, src3)

            # ---------------- phase 1: attention ----------------
            with (
                tc.tile_pool(name="ph1", bufs=1) as p1,
                tc.tile_pool(name="pe", bufs=3) as pe_pool,
                tc.tile_pool(name="pao", bufs=2) as pao,
                tc.tile_pool(name="pp_s", bufs=2, space="PSUM") as pp_s,
                tc.tile_pool(name="pp_a", bufs=2, space="PSUM") as pp_a,
            ):
                mt = p1.tile([128, NJ * N], bf)
                for jc in range(NJ):
                    nc.sync.dma_start(mt[:, N * jc:N * (jc + 1)],
                                      maskt[128 * jc:128 * (jc + 1), :])

                for pair in range(2):
                    q_off = N * pair       # qT01 at 0, qT23 at N
                    k_off = N * (2 + pair)
                    for ib2 in range(2):   # i blocks of 1024
                        accs = []
                        for hh in range(2):
                            acc = pp_a.tile([65, IB], f32, name=f"acc{hh}",
                                            tag="acc")
                            accs.append(acc)
                        for jc in range(NJ):
                            for hh in range(2):
                                hl = 2 * pair + hh  # local head 0..3
                                s_ps = pp_s.tile([128, IB], f32, name="s_ps",
                                                 tag="s_ps")
                                for ih in range(2):
                                    nc.tensor.matmul(
                                        s_ps[:, 512 * ih:512 * ih + 512],
                                        qkt[64 * hh:64 * hh + 64,
                                            k_off + 128 * jc:k_off + 128 * jc + 128],
                                        qkt[64 * hh:64 * hh + 64,
                                            q_off + IB * ib2 + 512 * ih:
                                            q_off + IB * ib2 + 512 * ih + 512],
                                        start=True, stop=True,
                                    )
                                e_t = pe_pool.tile([128, IB], bf, name="e_t",
                                                   tag="e_t")
                                nc.scalar.activation(
                                    e_t[:], s_ps[:],
                                    mybir.ActivationFunctionType.Exp)
                                p_t = pe_pool.tile([128, IB], bf, name="p_t",
                                                   tag="p_t")
                                nc.vector.tensor_mul(
                                    p_t[:], e_t[:],
                                    mt[:, N * jc + IB * ib2:N * jc + IB * ib2 + IB])
                                for ih in range(2):
                                    nc.tensor.matmul(
                                        accs[hh][:, 512 * ih:512 * ih + 512],
                                        v_aug[:, 260 * jc + 65 * hl:
                                              260 * jc + 65 * hl + 65],
                                        p_t[:, 512 * ih:512 * ih + 512],
                                        start=(jc == 0), stop=(jc == NJ - 1),
                                    )
                        for hh in range(2):
                            hl = 2 * pair + hh
                            ao = pao.tile([65, IB], bf, name="ao", tag="ao")
                            nc.vector.tensor_copy(ao[:], accs[hh][:])
                            nc.sync.dma_start(
                                ag_in[64 * hl:64 * hl + 64,
                                      IB * ib2:IB * ib2 + IB],
                                ao[0:64, :])
                            nc.sync.dma_start(
                                ag_in[256 + hl:257 + hl,
                                      IB * ib2:IB * ib2 + IB],
                                ao[64:65, :])

            nc.gpsimd.collective_compute(
                "AllGather",
                mybir.AluOpType.bypass,
                replica_groups=[[0, 1, 2, 3], [4, 5, 6, 7]],
                ins=[ag_in[:].opt()],
                outs=[ag_out[:].opt()],
            )
            if DEBUG:
                # fp32r bits are not fp32: cast on the way out via SWDGE
                nc.gpsimd.dma_start(dbg_qkt[:], qkt[:])
                nc.sync.dma_start(dbg_vaug[:], v_aug[:])
                nc.sync.dma_start(dbg_ag[:], ag_out[:])

            # ---------------- phase 2: normalize + output projection ------
            with (
                tc.tile_pool(name="ph2", bufs=1) as p2,
                tc.tile_pool(name="pp_bc", bufs=2, space="PSUM") as pp_bc,
                tc.tile_pool(name="pp_o", bufs=2, space="PSUM") as pp_o,
            ):
                pid = nc.sync.partition_id()
                i0 = (pid % 4) * 512

                wout_sb = p2.tile([128, 8 * D], bf)
                for k in range(8):
                    nc.sync.dma_start(wout_sb[:, D * k:D * (k + 1)],
                                      wout[128 * k:128 * (k + 1), :])
                e_sb = p2.tile([HEADS, D], bf)
                nc.sync.dma_start(e_sb[:], emat[:])

                sums_sb = p2.tile([HEADS, 512], bf)
                for r_i in range(4):
                    nc.sync.dma_start(
                        sums_sb[4 * r_i:4 * r_i + 4, :],
                        ag_out[260 * r_i + 256:260 * r_i + 260,
                               bass.ds(i0, 512)])
                rec = p2.tile([HEADS, 512], bf)
                with nc.allow_low_precision(reason="softmax denom recip bf16"):
                    nc.vector.reciprocal(rec[:], sums_sb[:])
                if DEBUG:
                    nc.sync.dma_start(dbg_sums[:], sums_sb[:])
                    nc.sync.dma_start(dbg_rec[:], rec[:])

                attn_raw = p2.tile([128, 8 * 512], bf)
                attn_n = p2.tile([128, 8 * 512], bf)
                for m in range(8):
                    r_i, half = divmod(m, 2)
                    nc.sync.dma_start(
                        attn_raw[:, 512 * m:512 * m + 512],
                        ag_out[260 * r_i + 128 * half:
                               260 * r_i + 128 * half + 128, bass.ds(i0, 512)])
                    bc = pp_bc.tile([128, 512], f32, name="bc", tag="bc")
                    nc.tensor.matmul(bc[:], e_sb[:, 128 * m:128 * m + 128],
                                     rec[:], start=True, stop=True)
                    nc.vector.tensor_mul(attn_n[:, 512 * m:512 * m + 512],
                                         attn_raw[:, 512 * m:512 * m + 512],
                                         bc[:])

                if DEBUG:
                    nc.sync.dma_start(dbg_attnn[:], attn_n[:])
                ost_pool = tc.tile_pool(name="ost", bufs=3)
                with ost_pool as po:
                    for mo in range(4):  # my-i chunks of 128
                        for nh in range(2):  # dout halves of 512
                            ps = pp_o.tile([128, 512], f32, name="ps_o",
                                           tag="ps_o")
                            for k in range(8):
                                nc.tensor.matmul(
                                    ps[:],
                                    attn_n[:, 512 * k + 128 * mo:
                                           512 * k + 128 * mo + 128],
                                    wout_sb[:, D * k + 512 * nh:
                                            D * k + 512 * nh + 512],
                                    start=(k == 0), stop=(k == 7),
                                )
                            ot = po.tile([128, 512], f32, name="ot", tag="ot")
                            nc.vector.tensor_copy(ot[:], ps[:])
                            nc.sync.dma_start(
                                out[128 * mo:128 * mo + 128,
                                    512 * nh:512 * nh + 512],
                                ot[:])

    nc.compile()
    return nc


def _get_nc():
    global _cached_nc
    if _cached_nc is None:
        _cached_nc = _build()
    return _cached_nc


def kernel(x, mask, W_qkv, W_out, b_out):
    x = np.asarray(x, dtype=np.float32)
    mask = np.asarray(mask)
    W_qkv = np.asarray(W_qkv, dtype=np.float32)
    W_out = np.asarray(W_out, dtype=np.float32)
    b_out = np.asarray(b_out, dtype=np.float32)

    nc = _get_nc()

    maskt_bf = np.ascontiguousarray(mask.reshape(N, N).T).astype(BF16)
    wout_bf = W_out.astype(BF16)
    emat = np.kron(np.eye(HEADS, dtype=np.float32),
                   np.ones((1, HD), dtype=np.float32)).astype(BF16)
    emat = np.ascontiguousarray(emat)

    in_maps = []
    for c in range(N_CORES):
        b = c // 4
        g = c % 4
        hs = slice(g * HPC * HD, (g + 1) * HPC * HD)  # 256 cols of this core
        wq = W_qkv[:, 0 * D:1 * D][:, hs] * np.float32(SCALE)
        wk = W_qkv[:, 1 * D:2 * D][:, hs]
        wv = W_qkv[:, 2 * D:3 * D][:, hs]
        wqkv_c = np.ascontiguousarray(
            np.concatenate([wq, wk, wv], axis=1), dtype=np.float32)
        xt_c = np.ascontiguousarray(x[b].T)
        in_maps.append({
            "xt": xt_c,
            "wqkv": wqkv_c,
            "maskt": maskt_bf,
            "wout": wout_bf,
            "emat": emat,
        })

    global _last_in_maps, _last_res
    _last_in_maps = in_maps

    res = bass_utils.run_bass_kernel_spmd(
        nc, in_maps, core_ids=list(range(N_CORES)))
    _last_res = res

    out_full = np.empty((B, N, D), dtype=np.float32)
    for c in range(N_CORES):
        b = c // 4
        g = c % 4
        out_full[b, 512 * g:512 * (g + 1), :] = res.results[c]["out"]
    out_full += b_out
    return out_full


# revision 22
# speedup vs baseline: 1.1459x; 1.1459x over previous
"""Distributed Trainium2 (8 NeuronCores) kernel for nn_AdaptiveAttention.

Reference computation (b=2, n=2048, d=1024, 16 heads x 64):
    qkv = x @ W_qkv; q,k,v = split(qkv)
    attn = softmax(mask(q k^T / sqrt(dh)))
    out  = (attn @ v) @ W_out + b_out

Sharding: core c in [0,8) handles batch b = c//4 and head group g = c%4
(heads 4g..4g+3).  Data parallel over b, tensor parallel over heads.
The output projection needs all heads, so the per-head attention outputs
(plus softmax denominators) are AllGather'd (bf16, groups of 4) and each
core computes the output projection for its quarter of the sequence,
selected with a dynamic (partition-id-derived) slice.

Numerics: projections and scores run the TensorEngine in float32r
(fp32 storage, ~1e-4 matmul error, full PE rate); exp output, mask and
the P = exp(s)*mask matrix are bf16; all matmuls accumulate in fp32.
Softmax runs without max-subtraction (scores are O(1) by construction)
as exp(s) * mask, with row sums obtained by augmenting v with a ones
column; normalization happens after the AllGather via a reciprocal and
a small expansion matmul.
"""

import numpy as np
import ml_dtypes

import concourse.bass as bass
import concourse.tile as tile
from concourse import bacc, mybir
from concourse import bass_utils

BF16 = ml_dtypes.bfloat16

B = 2
N = 2048
D = 1024
HEADS = 16
HD = 64  # head dim
SCALE = HD ** -0.5
N_CORES = 8
HPC = 4  # heads per core
IB = 1024  # i-block size in attention inner tiles
NJ = N // 128  # 16 j-chunks

_cached_nc = None
_last_in_maps = None
_last_res = None
DEBUG = False


def _build():
    nc = bacc.Bacc("TRN2", target_bir_lowering=False, debug=False,
                   num_devices=N_CORES)

    f32 = mybir.dt.float32
    f32r = mybir.dt.float32r
    bf = mybir.dt.bfloat16

    xt = nc.dram_tensor("xt", [D, N], bf, kind="ExternalInput")
    wqkv = nc.dram_tensor("wqkv", [D, 768], bf, kind="ExternalInput")
    maskt = nc.dram_tensor("maskt", [N, N], bf, kind="ExternalInput")
    wout = nc.dram_tensor("wout", [D, D], bf, kind="ExternalInput")
    emat = nc.dram_tensor("emat", [HEADS, D], bf, kind="ExternalInput")
    out = nc.dram_tensor("out", [N // 4, D], f32, kind="ExternalOutput")
    if DEBUG:
        dbg_qkt = nc.dram_tensor("dbg_qkt", [128, 4 * N], bf,
                                 kind="ExternalOutput")
        dbg_vaug = nc.dram_tensor("dbg_vaug", [128, NJ * 260], bf,
                                  kind="ExternalOutput")
        dbg_ag = nc.dram_tensor("dbg_ag", [8 * 4 * 65, N], bf,
                                kind="ExternalOutput")
        dbg_sums = nc.dram_tensor("dbg_sums", [HEADS, 512], bf,
                                  kind="ExternalOutput")
        dbg_rec = nc.dram_tensor("dbg_rec", [HEADS, 512], bf,
                                 kind="ExternalOutput")
        dbg_attnn = nc.dram_tensor("dbg_attnn", [128, 8 * 512], bf,
                                   kind="ExternalOutput")

    with tile.TileContext(nc) as tc:
        with (
            tc.tile_pool(name="res", bufs=1) as res,
            tc.tile_pool(name="dram", bufs=1, space="DRAM") as dram,
        ):
            # resident tensors
            # qkt: [qT01 | qT23 | kT01 | kT23], each [128, 2048] bf16
            qkt = res.tile([128, 4 * N], bf)
            # v_aug: per j-chunk jc block of 260 cols: 4x(64 v cols + ones)
            v_aug = res.tile([128, NJ * 260], bf)

            ag_in = dram.tile([4 * 65, N], bf)       # rows: 4x64 attn + 4 sums
            ag_out = dram.tile([8 * 4 * 65, N], bf)  # gathered over all 8 ranks

            # ---------------- phase 0: load + projections ----------------
            with (
                tc.tile_pool(name="ph0", bufs=1) as p0,
                tc.tile_pool(name="pp_qk", bufs=4, space="PSUM") as pp_qk,
                tc.tile_pool(name="pp_v", bufs=2, space="PSUM") as pp_v,
            ):
                xtr = p0.tile([128, 8 * N], bf)
                wr = p0.tile([128, 8 * 768], bf)
                for k in range(8):
                    nc.sync.dma_start(xtr[:, N * k:N * (k + 1)],
                                      xt[128 * k:128 * (k + 1), :])
                    nc.sync.dma_start(wr[:, 768 * k:768 * (k + 1)],
                                      wqkv[128 * k:128 * (k + 1), :])

                nc.vector.memset(v_aug[:], 1.0)

                # qT / kT stacked pairs: wcol selects the 128 W columns
                for t_i in range(4):  # qT01, qT23, kT01, kT23
                    wcol = 128 * t_i
                    for nb in range(4):  # n blocks of 512
                        ps = pp_qk.tile([128, 512], f32, name="ps_qk", tag="ps_qk")
                        for k in range(8):
                            nc.tensor.matmul(
                                ps[:],
                                wr[:, 768 * k + wcol:768 * k + wcol + 128],
                                xtr[:, N * k + 512 * nb:N * k + 512 * nb + 512],
                                start=(k == 0), stop=(k == 7),
                            )
                        nc.vector.tensor_copy(
                            qkt[:, N * t_i + 512 * nb:N * t_i + 512 * nb + 512],
                            ps[:])

                # v: natural layout [j, 4*64], copied into v_aug with ones col
                for jc in range(NJ):
                    ps = pp_v.tile([128, 256], f32, name="ps_v", tag="ps_v")
                    for k in range(8):
                        nc.tensor.matmul(
                            ps[:],
                            xtr[:, N * k + 128 * jc:N * k + 128 * jc + 128],
                            wr[:, 768 * k + 512:768 * k + 768],
                            start=(k == 0), stop=(k == 7),
                        )
                    dst = v_aug[:, 260 * jc:260 * jc + 260]
                    dst3 = dst.rearrange("p (h c) -> p h c", h=4, c=65)[:, :, 0:64]
                    src3 = ps[:].rearrange("p (h c) -> p h c", h=4, c=64)
                    nc.vector.tensor_copy(dst3, src3)

            # ---------------- phase 1: attention ----------------
            with (
                tc.tile_pool(name="ph1", bufs=1) as p1,
                tc.tile_pool(name="pe", bufs=3) as pe_pool,
                tc.tile_pool(name="pao", bufs=2) as pao,
                tc.tile_pool(name="pp_s", bufs=2, space="PSUM") as pp_s,
                tc.tile_pool(name="pp_a", bufs=2, space="PSUM") as pp_a,
            ):
                mt = p1.tile([128, NJ * N], bf)
                for jc in range(NJ):
                    nc.sync.dma_start(mt[:, N * jc:N * (jc + 1)],
                                      maskt[128 * jc:128 * (jc + 1), :])

                for pair in range(2):
                    q_off = N * pair       # qT01 at 0, qT23 at N
                    k_off = N * (2 + pair)
                    for ib2 in range(2):   # i blocks of 1024
                        accs = []
                        for hh in range(2):
                            acc = pp_a.tile([65, IB], f32, name=f"acc{hh}",
                                            tag="acc")
                            accs.append(acc)
                        for jc in range(NJ):
                            for hh in range(2):
                                hl = 2 * pair + hh  # local head 0..3
                                s_ps = pp_s.tile([128, IB], f32, name="s_ps",
                                                 tag="s_ps")
                                for ih in range(2):
                                    nc.tensor.matmul(
                                        s_ps[:, 512 * ih:512 * ih + 512],
                                        qkt[64 * hh:64 * hh + 64,
                                            k_off + 128 * jc:k_off + 128 * jc + 128],
                                        qkt[64 * hh:64 * hh + 64,
                                            q_off + IB * ib2 + 512 * ih:
                                            q_off + IB * ib2 + 512 * ih + 512],
                                        start=True, stop=True,
                                    )
                                e_t = pe_pool.tile([128, IB], bf, name="e_t",
                                                   tag="e_t")
                                nc.scalar.activation(
                                    e_t[:], s_ps[:],
                                    mybir.ActivationFunctionType.Exp)
                                p_t = pe_pool.tile([128, IB], bf, name="p_t",
                                                   tag="p_t")
                                nc.vector.tensor_mul(
                                    p_t[:], e_t[:],
                                    mt[:, N * jc + IB * ib2:N * jc + IB * ib2 + IB])
                                for ih in range(2):
                                    nc.tensor.matmul(
                                        accs[hh][:, 512 * ih:512 * ih + 512],
                                        v_aug[:, 260 * jc + 65 * hl:
                                              260 * jc + 65 * hl + 65],
                                        p_t[:, 512 * ih:512 * ih + 512],
                                        start=(jc == 0), stop=(jc == NJ - 1),
                                    )
                        for hh in range(2):
                            hl = 2 * pair + hh
                            ao = pao.tile([65, IB], bf, name="ao", tag="ao")
                            nc.vector.tensor_copy(ao[:], accs[hh][:])
                            nc.sync.dma_start(
                                ag_in[64 * hl:64 * hl + 64,
                                      IB * ib2:IB * ib2 + IB],
                                ao[0:64, :])
                            nc.sync.dma_start(
                                ag_in[256 + hl:257 + hl,
                                      IB * ib2:IB * ib2 + IB],
                                ao[64:65, :])

            nc.gpsimd.collective_compute(
                "AllGather",
                mybir.AluOpType.bypass,
                replica_groups=[[0, 1, 2, 3, 4, 5, 6, 7]],
                ins=[ag_in[:].opt()],
                outs=[ag_out[:].opt()],
            )
            if DEBUG:
                nc.sync.dma_start(dbg_qkt[:], qkt[:])
                nc.sync.dma_start(dbg_vaug[:], v_aug[:])
                nc.sync.dma_start(dbg_ag[:], ag_out[:])

            # ---------------- phase 2: normalize + output projection ------
            with (
                tc.tile_pool(name="ph2", bufs=1) as p2,
                tc.tile_pool(name="pp_bc", bufs=2, space="PSUM") as pp_bc,
                tc.tile_pool(name="pp_o", bufs=2, space="PSUM") as pp_o,
            ):
                pid = nc.sync.partition_id()
                i0 = (pid % 4) * 512
                goff = (pid // 4) * 1040  # my batch group's block in ag_out

                wout_sb = p2.tile([128, 8 * D], bf)
                for k in range(8):
                    nc.sync.dma_start(wout_sb[:, D * k:D * (k + 1)],
                                      wout[128 * k:128 * (k + 1), :])
                e_sb = p2.tile([HEADS, D], bf)
                nc.sync.dma_start(e_sb[:], emat[:])

                sums_sb = p2.tile([HEADS, 512], bf)
                for r_i in range(4):
                    nc.sync.dma_start(
                        sums_sb[4 * r_i:4 * r_i + 4, :],
                        ag_out[bass.ds(goff + 260 * r_i + 256, 4),
                               bass.ds(i0, 512)])
                rec = p2.tile([HEADS, 512], bf)
                with nc.allow_low_precision(reason="softmax denom recip bf16"):
                    nc.vector.reciprocal(rec[:], sums_sb[:])
                if DEBUG:
                    nc.sync.dma_start(dbg_sums[:], sums_sb[:])
                    nc.sync.dma_start(dbg_rec[:], rec[:])

                attn_raw = p2.tile([128, 8 * 512], bf)
                attn_n = p2.tile([128, 8 * 512], bf)
                for m in range(8):
                    r_i, half = divmod(m, 2)
                    nc.sync.dma_start(
                        attn_raw[:, 512 * m:512 * m + 512],
                        ag_out[bass.ds(goff + 260 * r_i + 128 * half, 128),
                               bass.ds(i0, 512)])
                    bc = pp_bc.tile([128, 512], f32, name="bc", tag="bc")
                    nc.tensor.matmul(bc[:], e_sb[:, 128 * m:128 * m + 128],
                                     rec[:], start=True, stop=True)
                    nc.vector.tensor_mul(attn_n[:, 512 * m:512 * m + 512],
                                         attn_raw[:, 512 * m:512 * m + 512],
                                         bc[:])

                if DEBUG:
                    nc.sync.dma_start(dbg_attnn[:], attn_n[:])
                ost_pool = tc.tile_pool(name="ost", bufs=3)
                with ost_pool as po:
                    for mo in range(4):  # my-i chunks of 128
                        for nh in range(2):  # dout halves of 512
                            ps = pp_o.tile([128, 512], f32, name="ps_o",
                                           tag="ps_o")
                            for k in range(8):
                                nc.tensor.matmul(
                                    ps[:],
                                    attn_n[:, 512 * k + 128 * mo:
                                           512 * k + 128 * mo + 128],
                                    wout_sb[:, D * k + 512 * nh:
                                            D * k + 512 * nh + 512],
                                    start=(k == 0), stop=(k == 7),
                                )
                            ot = po.tile([128, 512], f32, name="ot", tag="ot")
                            nc.vector.tensor_copy(ot[:], ps[:])
                            nc.sync.dma_start(
                                out[128 * mo:128 * mo + 128,
                                    512 * nh:512 * nh + 512],
                                ot[:])

    nc.compile()
    return nc


def _get_nc():
    global _cached_nc
    if _cached_nc is None:
        _cached_nc = _build()
    return _cached_nc


def kernel(x, mask, W_qkv, W_out, b_out):
    x = np.asarray(x, dtype=np.float32)
    mask = np.asarray(mask)
    W_qkv = np.asarray(W_qkv, dtype=np.float32)
    W_out = np.asarray(W_out, dtype=np.float32)
    b_out = np.asarray(b_out, dtype=np.float32)

    nc = _get_nc()

    maskt_bf = np.ascontiguousarray(mask.reshape(N, N).T).astype(BF16)
    wout_bf = W_out.astype(BF16)
    emat = np.kron(np.eye(HEADS, dtype=np.float32),
                   np.ones((1, HD), dtype=np.float32)).astype(BF16)
    emat = np.ascontiguousarray(emat)

    in_maps = []
    for c in range(N_CORES):
        b = c // 4
        g = c % 4
        hs = slice(g * HPC * HD, (g + 1) * HPC * HD)  # 256 cols of this core
        wq = W_qkv[:, 0 * D:1 * D][:, hs] * np.float32(SCALE)
        wk = W_qkv[:, 1 * D:2 * D][:, hs]
        wv = W_qkv[:, 2 * D:3 * D][:, hs]
        wqkv_c = np.ascontiguousarray(
            np.concatenate([wq, wk, wv], axis=1)).astype(BF16)
        xt_c = np.ascontiguousarray(x[b].T).astype(BF16)
        in_maps.append({
            "xt": xt_c,
            "wqkv": wqkv_c,
            "maskt": maskt_bf,
            "wout": wout_bf,
            "emat": emat,
        })

    global _last_in_maps, _last_res
    _last_in_maps = in_maps

    res = bass_utils.run_bass_kernel_spmd(
        nc, in_maps, core_ids=list(range(N_CORES)))
    _last_res = res

    out_full = np.empty((B, N, D), dtype=np.float32)
    for c in range(N_CORES):
        b = c // 4
        g = c % 4
        out_full[b, 512 * g:512 * (g + 1), :] = res.results[c]["out"]
    out_full += b_out
    return out_full
